# revision 70
# baseline (speedup 1.0000x reference)
"""Distributed Bass/Tile kernel for EnhancedDecoderAttention on 8 Trainium2 cores.

Module: q/k/v projections (+bias), rotate-halves RoPE on q/k, causal
masked softmax attention, output projection (+bias).
Shapes: x [4, 2048, 1024], 16 heads, head_dim 64.

Sharding: core c handles batch b = c//2 and head-half hh = c%2
(8 of 16 heads), i.e. column-sharded Wq/Wk/Wv, row-sharded Wo;
per-core partial outputs are summed pairwise on the host.

Design (v1 measured 461 us/iter on this harness; this version 345 us).
Key measured HW facts driving the design (microbenches in microbench.py):
  - ACT exp [128,1024] = 962ns; PE matmul N=512 = 180ns (stationary
    rotation is free); DVE SBUF-SBUF mul [128,512] = 139ns, but ANY
    PSUM-reading DVE op is ~590-770ns; Pool TensorTensor is 1043ns;
    nc.vector.reciprocal is ~3.5us per call (5x the sim model!) and
    gpsimd.partition_broadcast ~860ns.  The v1 softmax-normalize chain
    (2 reciprocals + 2 partition_broadcasts + muls per pair-chunk) cost
    ~167us/iter of the 461.
Structure:
  - projections and out-projections are emitted as small "feeder units"
    (4 matmuls) interleaved between attention tile-steps, so PE-heavy
    projection work fills PE stalls while ACT (exp) paces the attention
    chain (the TileScheduler's priority is emission order; with all
    projection emitted up front, ACT idles the whole projection phase).
    Only chunk 0's q/k/v projection runs up front.
  - attention processed per (head-PAIR, sq-chunk-of-512): the two heads
    of a pair live at partitions 0-63 / 64-127 of qh/kh, so their score
    matmuls run concurrently on disjoint PE row-groups (K=64 each).
  - causal masking of the diagonal 128x128 block is done ON PE: an extra
    matmul (strict-upper-triangle -400 constant x identity) accumulates
    a -400 bias into the masked positions before the exp, so exp gives
    ~0 there.  No DVE mask-multiply, no chain hop.
  - diagonal-tile exps use one 3D-AP activation call covering both
    heads' valid column ranges ([p, 2, n] with stride 512).
  - AV matmuls are emitted with a 2-tile delay queue so exp(t) has two
    tile-periods of slack before PE needs ex(t).
  - softmax normalize with NO reciprocal and NO partition broadcast:
    v carries 64 ones-columns (VS=130), so the AV matmul (M=128, same
    cycle cost as M=65 since cost = N) replicates the denominator into
    PSUM rows 64-127; 1/den = exp(-ln(den)) via two ACT spline calls
    (ln and exp share the natural_log_exp_and_others table set, so no
    table switching), then two DVE muls finish both heads.
"""

import numpy as np
import ml_dtypes
from collections import deque
from contextlib import ExitStack

import concourse.bass as bass
import concourse.tile as tile
from concourse import bacc, mybir
from concourse.bass_utils import run_bass_kernel_spmd

BF = mybir.dt.bfloat16
F32 = mybir.dt.float32
AF = mybir.ActivationFunctionType

B, S, E, H, D = 4, 2048, 1024, 16, 64
NCORE = 8
HL = H // 2          # 8 local heads
NPAIR = HL // 2      # 4 local head pairs
EL = HL * D          # 512 local e_out
KE = E // 128        # 8 e_in tiles
NT = S // 128        # 16 sk tiles
NCH = S // 512       # 4 sq chunks
VS = 130             # v_s per-head stride (64 d + 64 ones + 2 pad); the 64
                     # ones-columns make AV (M=128) replicate the softmax
                     # denominator into PSUM rows 64-127 for free
NEG = -400.0         # causal bias: exp(0.125*(s-400)) == 0 in bf16

_PROG_CACHE = {}
ABLATE = set()       # timing-bisection flags used by bench_ablate.py


def _emit_body(nc, tc, ctx, aps, variant):
    causal = variant == "causal"
    masked = variant == "masked"

    per = ctx.enter_context(tc.tile_pool(name="per", bufs=1))
    prep = ctx.enter_context(tc.tile_pool(name="prep", bufs=4))
    ropp = ctx.enter_context(tc.tile_pool(name="ropp", bufs=2))
    expp = ctx.enter_context(tc.tile_pool(name="expp", bufs=8))
    outp = ctx.enter_context(tc.tile_pool(name="outp", bufs=4))
    smallp = ctx.enter_context(tc.tile_pool(name="smallp", bufs=2))
    pp = ctx.enter_context(tc.tile_pool(name="pp", bufs=2, space="PSUM"))
    pa = ctx.enter_context(tc.tile_pool(name="pa", bufs=2, space="PSUM"))
    pb = ctx.enter_context(tc.tile_pool(name="pb", bufs=1, space="PSUM"))
    if masked:
        mtp = ctx.enter_context(tc.tile_pool(name="mtp", bufs=4))

    # ---- persistent loads: xt chunk 0 + q/k weights first, rest follows ----
    xt_sb = per.tile([128, KE, S], BF)
    nc.sync.dma_start(
        xt_sb[:, :, 0:512],
        aps["xt"][:, 0:512].rearrange("(k p) s -> p k s", p=128))
    wq_sb = per.tile([128, KE, EL], BF)
    nc.sync.dma_start(wq_sb[:], aps["wq"].rearrange("(k p) n -> p k n", p=128))
    bq_sb = per.tile([128, 4], F32)
    nc.sync.dma_start(bq_sb[:], aps["bq"].rearrange("(m p) -> p m", p=128))
    wk_sb = per.tile([128, KE, EL], BF)
    nc.sync.dma_start(wk_sb[:], aps["wk"].rearrange("(k p) n -> p k n", p=128))
    bk_sb = per.tile([128, 4], F32)
    nc.sync.dma_start(bk_sb[:], aps["bk"].rearrange("(m p) -> p m", p=128))
    cos_sb = per.tile([128, S], BF)
    nc.sync.dma_start(cos_sb[:], aps["cos4"][:])
    sin_sb = per.tile([128, S], BF)
    nc.sync.dma_start(sin_sb[:], aps["sin4"][:])
    wv_sb = per.tile([128, KE, EL], BF)
    nc.sync.dma_start(wv_sb[:], aps["wv"].rearrange("(k p) n -> p k n", p=128))
    bv_sb = per.tile([128, EL], BF)
    nc.sync.dma_start(bv_sb[:], aps["bv_bc"][:])
    if causal:
        utneg_sb = per.tile([128, 128], BF)
        nc.sync.dma_start(utneg_sb[:], aps["utneg"][:])
        ident_sb = per.tile([128, 128], BF)
        nc.sync.dma_start(ident_sb[:], aps["ident"][:])
    for c in range(1, NCH):
        nc.sync.dma_start(
            xt_sb[:, :, c * 512:(c + 1) * 512],
            aps["xt"][:, c * 512:(c + 1) * 512].rearrange("(k p) s -> p k s",
                                                          p=128))
    wo_sb = per.tile([128, 4, E], BF)
    nc.sync.dma_start(wo_sb[:], aps["wo"].rearrange("(k p) n -> p k n", p=128))

    qh_sb = [per.tile([128, S], BF, name=f"qh{i}") for i in range(NPAIR)]
    kh_sb = [per.tile([128, S], BF, name=f"kh{i}") for i in range(NPAIR)]
    vs_sb = [per.tile([128, HL, VS], BF, name=f"vs{i}") for i in range(NT)]
    anT_sb = [per.tile([128, S], BF, name=f"anT{i}") for i in range(NPAIR)]
    if "avdep" in ABLATE:
        exc_sb = per.tile([128, 1024], BF, name="exc")
        nc.vector.memset(exc_sb[:], 0.001)
    if "attn" in ABLATE or "fastevac" in ABLATE:
        for _p in range(NPAIR):
            nc.vector.memset(anT_sb[_p][:], 0.001)
    if "noproj" in ABLATE:
        for _p in range(NPAIR):
            nc.vector.memset(qh_sb[_p][:], 0.01)
            nc.vector.memset(kh_sb[_p][:], 0.01)
        for _t in range(NT):
            nc.vector.memset(vs_sb[_t][:], 0.01)
    junk_sb = per.tile([65, 512], F32, name="junk")

    bv3 = bv_sb[:].rearrange("p (h d) -> p h d", d=D)
    pend_ps = {}
    pend_pre = {}

    # ---- feeder units: closures each emitting ~850ns of PE work ----
    def u_projqk(which, m, c, half):
        def emit():
            w_sb = wq_sb if which == "q" else wk_sb
            if half == 0:
                ps = pp.tile([128, 512], F32, tag="ps", name="ps_qk")
                pend_ps[(which, m, c)] = ps
            else:
                ps = pend_ps.pop((which, m, c))
            for ki in range(4 * half, 4 * half + 4):
                nc.tensor.matmul(ps[:], w_sb[:, ki, m * 128:(m + 1) * 128],
                                 xt_sb[:, ki, c * 512:(c + 1) * 512],
                                 start=(ki == 0), stop=(ki == KE - 1))
            if half == 1:
                b_sb = bq_sb if which == "q" else bk_sb
                pre = prep.tile([128, 512], BF, tag="pre", name="pre")
                pend_pre[(which, m // 2, m % 2)] = pre
                nc.vector.tensor_scalar_add(pre[:], ps[:], b_sb[:, m:m + 1])
        return emit

    def u_rope(which, g, c):
        def emit():
            dsts = qh_sb if which == "q" else kh_sb
            p0 = pend_pre.pop((which, g, 0))
            p1 = pend_pre.pop((which, g, 1))
            cs = cos_sb[:, c * 512:(c + 1) * 512]
            sn = sin_sb[:, c * 512:(c + 1) * 512]
            tmp = ropp.tile([128, 512], BF, tag="tmp", name="tmp")
            tmp2 = ropp.tile([128, 512], BF, tag="tmp", name="tmp2")
            rr = ropp.tile([128, 512], BF, tag="rot", name="rr")
            ri = ropp.tile([128, 512], BF, tag="rot", name="ri")
            nc.gpsimd.tensor_mul(tmp[:], p1[:], sn)
            nc.vector.tensor_mul(rr[:], p0[:], cs)
            nc.vector.tensor_sub(rr[:], rr[:], tmp[:])
            nc.gpsimd.tensor_mul(tmp2[:], p0[:], sn)
            nc.vector.tensor_mul(ri[:], p1[:], cs)
            nc.vector.tensor_add(ri[:], ri[:], tmp2[:])
            for hq in range(4):
                h = 4 * g + hq
                pair, off = h // 2, 64 * (h % 2)
                nc.sync.dma_start(dsts[pair][off:off + 32,
                                             c * 512:(c + 1) * 512],
                                  rr[hq * 32:(hq + 1) * 32, :])
                nc.sync.dma_start(dsts[pair][off + 32:off + 64,
                                             c * 512:(c + 1) * 512],
                                  ri[hq * 32:(hq + 1) * 32, :])
        return emit

    def u_projv(t, half):
        def emit():
            if half == 0:
                ps = pp.tile([128, 512], F32, tag="ps", name="ps_v")
                pend_ps[("v", t)] = ps
            else:
                ps = pend_ps.pop(("v", t))
            for ki in range(4 * half, 4 * half + 4):
                nc.tensor.matmul(ps[:], xt_sb[:, ki, t * 128:(t + 1) * 128],
                                 wv_sb[:, ki, :], start=(ki == 0),
                                 stop=(ki == KE - 1))
            if half == 1:
                nc.vector.memset(vs_sb[t][:, :, D:2 * D], 1.0)
                nc.vector.tensor_add(vs_sb[t][:, :, 0:D],
                                     ps[:].rearrange("p (h d) -> p h d", d=D),
                                     bv3)
        return emit

    def u_outproj(c, et):
        def emit():
            ps = pp.tile([128, 512], F32, tag="ps", name="ps_o")
            for pi in range(NPAIR):
                nc.tensor.matmul(ps[:], wo_sb[:, pi, et * 128:(et + 1) * 128],
                                 anT_sb[pi][:, c * 512:(c + 1) * 512],
                                 start=(pi == 0), stop=(pi == NPAIR - 1))
            ot = outp.tile([128, 512], BF, tag="ot", name="ot")
            # Pool cannot read PSUM; DVE carries this
            nc.vector.tensor_copy(ot[:], ps[:])
            nc.sync.dma_start(
                aps["o"][et * 128:(et + 1) * 128, c * 512:(c + 1) * 512],
                ot[:])
        return emit

    def proj_units_for_chunk(c):
        us = []
        for g in (0, 1):
            for which in ("q", "k"):
                for part in (0, 1):
                    m = 2 * g + part
                    us.append(u_projqk(which, m, c, 0))
                    us.append(u_projqk(which, m, c, 1))
                us.append(u_rope(which, g, c))
            for t in (4 * c + 2 * g, 4 * c + 2 * g + 1):
                us.append(u_projv(t, 0))
                us.append(u_projv(t, 1))
        return us

    # ---- attention for one (pair, sq-chunk-of-512), AV delayed 2 tiles ----
    def attn_pair_chunk(pair, c, feed):
        A, Bh = 2 * pair, 2 * pair + 1
        qh, kh = qh_sb[pair], kh_sb[pair]
        t_hi = 4 * (c + 1) if causal else NT
        c0, c1 = c * 512, (c + 1) * 512
        # one tile, 2 banks: head A cols 0-511, head B cols 512-1023;
        # rows 0-63 numerators, row 64 denominators (ones-row of v),
        # rows 64-127 later overwritten by the broadcast reciprocal.
        psb = pb.tile([128, 1024], F32, tag="psb", name=f"psb{pair}{c}")
        t_last = t_hi - 1
        inflight = deque()

        def emit_av(t, lo, ex):
            if "avdep" in ABLATE:  # timing bisect: break the exp->AV edge
                ex = exc_sb
            last = (t == t_last)
            nc.tensor.matmul(psb[:, lo:512], vs_sb[t][:, A, 0:128],
                             ex[:, lo:512], start=(t == 0), stop=last)
            nc.tensor.matmul(psb[:, 512 + lo:1024], vs_sb[t][:, Bh, 0:128],
                             ex[:, 512 + lo:1024], start=(t == 0), stop=last)

        for t in range(t_hi):
            diag = causal and (t // 4 == c)
            lo = 128 * (t % 4) if diag else 0
            tl = t * 128
            ps = pa.tile([128, 1024], F32, tag="psa", name="ps_s")
            nc.tensor.matmul(ps[:, lo:512], kh[0:64, tl:tl + 128],
                             qh[0:64, c0 + lo:c1], start=True,
                             stop=not diag)
            nc.tensor.matmul(ps[:, 512 + lo:1024], kh[64:128, tl:tl + 128],
                             qh[64:128, c0 + lo:c1], start=True,
                             stop=not diag)
            if diag:
                # accumulate -400 into the masked (strict lower r>q) positions
                # of the 128-wide diagonal block, both heads
                nc.tensor.matmul(ps[:, lo:lo + 128], utneg_sb[:], ident_sb[:],
                                 start=False, stop=True)
                nc.tensor.matmul(ps[:, 512 + lo:512 + lo + 128], utneg_sb[:],
                                 ident_sb[:], start=False, stop=True)
            ex = expp.tile([128, 1024], BF, tag="ex", name="ex")
            if "exp" in ABLATE:  # timing bisect: cheap DVE fill instead of ACT
                nc.vector.memset(ex[:], 0.001)
                nc.vector.tensor_copy(ex[:, 0:64], ps[:, 0:64])
            elif lo:
                ex3 = ex[:].rearrange("p (h n) -> p h n", h=2)
                ps3 = ps[:].rearrange("p (h n) -> p h n", h=2)
                nc.scalar.activation(ex3[:, :, lo:512], ps3[:, :, lo:512],
                                     AF.Exp, scale=0.125)
            else:
                nc.scalar.activation(ex[:], ps[:], AF.Exp, scale=0.125)
            if masked:
                mt = mtp.tile([128, 512], BF, tag="mt", name="mt")
                nc.sync.dma_start(mt[:], aps["mt"][tl:tl + 128, c0:c1])
                nc.vector.tensor_mul(ex[:, 0:512], ex[:, 0:512], mt[:])
                nc.vector.tensor_mul(ex[:, 512:1024], ex[:, 512:1024], mt[:])
            inflight.append((t, lo, ex))
            feed()
            if "av" not in ABLATE:
                if len(inflight) > 7:
                    emit_av(*inflight.popleft())
        if "av" not in ABLATE:
            while inflight:
                emit_av(*inflight.popleft())
        return psb

    # softmax normalize with NO partition broadcast at all: AV's 64
    # ones-columns already replicated the denominator into psb rows 64-127,
    # so one DVE reciprocal [64,1024] + two DVE muls finish both heads.
    # head A lands in anT directly; head B goes through a partition-shift
    # DMA into anT rows 64-127.
    def evac_pair(pair, c, psb):
        if "fastevac" in ABLATE:  # timing bisect: free psb with cheap copies
            nc.vector.tensor_copy(junk_sb[:], psb[0:65, 0:512])
            nc.vector.tensor_copy(junk_sb[:], psb[0:65, 512:1024])
            return
        c0, c1 = c * 512, (c + 1) * 512
        # 1/den = exp(-ln(den)): two ACT spline passes (ln+exp share one
        # table set).  Alternatives measured: nc.vector.reciprocal costs
        # ~7us per [64,1024] on HW; reciprocal_approx_accurate would save
        # ~22us/iter by keeping the evac off ACT (the attention pacer) but
        # computes GARBAGE on this HW (rel err 3.0) — do not use.
        lnd = smallp.tile([64, 1024], F32, tag="lnd", name="lnd")
        nc.scalar.activation(lnd[:], psb[64:128, :], AF.Ln)
        rep = smallp.tile([64, 1024], F32, tag="rep", name="rep")
        nc.scalar.activation(rep[:], lnd[:], AF.Exp, scale=-1.0)
        nc.vector.tensor_mul(anT_sb[pair][0:64, c0:c1], psb[0:64, 0:512],
                             rep[:, 0:512])
        anstB = smallp.tile([64, 512], BF, tag="anst", name="anstB")
        nc.vector.tensor_mul(anstB[:], psb[0:64, 512:1024], rep[:, 512:1024])
        nc.sync.dma_start(anT_sb[pair][64:128, c0:c1], anstB[:])

    # ---- schedule: chunk-0 projection up front, then attention chunks
    # with next-chunk projection + prev-chunk out-projection interleaved ----
    carry = []
    if "noproj" not in ABLATE:
        su = proj_units_for_chunk(0)
        # up front: only what attention (pairs 0-1, chunk 0) needs — group
        # 0's q/k/rope (su[0:14]) plus group 1's v tiles t2,t3 (su[24:28]).
        # group 1's q/k/rope units ride at the head of window 0's feeder;
        # pairs 2-3 start ~8 tile-steps in, by which time they are done.
        for u in su[0:14] + su[24:28]:
            u()
        carry = su[14:24]
        if "feeder" in ABLATE:  # v1-style: all projections up front
            for u in carry:
                u()
            carry = []
            for cc in range(1, NCH):
                for u in proj_units_for_chunk(cc):
                    u()

    for c in range(NCH):
        units = deque(carry if c == 0 else ())
        if c + 1 < NCH and "feeder" not in ABLATE and "noproj" not in ABLATE:
            units.extend(proj_units_for_chunk(c + 1))
        if c >= 1 and "outproj" not in ABLATE:
            # outproj(c-1) directly after attn chunk c-1: measured better
            # (344us) than deferring outproj into the PE-starved window 3
            # (364us) — the early windows need the fill despite the model
            units.extend(u_outproj(c - 1, et) for et in range(KE))
        steps = NPAIR * (4 * (c + 1) if causal else NT)
        per_step = len(units) / steps if steps else 0.0
        acc = 0.0

        def feed():
            nonlocal acc
            acc += per_step
            while acc >= 1.0 and units:
                acc -= 1.0
                units.popleft()()

        if "attn" not in ABLATE:
            for pair in range(NPAIR):
                psb = attn_pair_chunk(pair, c, feed)
                if "evac" not in ABLATE:
                    evac_pair(pair, c, psb)
        while units:
            units.popleft()()
    if "outproj" not in ABLATE:
        for et in range(KE):
            u_outproj(NCH - 1, et)()


def _build_program(variant, reps=1):
    key = (variant, reps)
    if key in _PROG_CACHE:
        return _PROG_CACHE[key]
    nc = bacc.Bacc("TRN2", target_bir_lowering=False, debug=False,
                   num_devices=NCORE)
    aps = {
        "xt": nc.dram_tensor("xt", [E, S], BF, kind="ExternalInput").ap(),
        "wq": nc.dram_tensor("wq", [E, EL], BF, kind="ExternalInput").ap(),
        "wk": nc.dram_tensor("wk", [E, EL], BF, kind="ExternalInput").ap(),
        "wv": nc.dram_tensor("wv", [E, EL], BF, kind="ExternalInput").ap(),
        "wo": nc.dram_tensor("wo", [EL, E], BF, kind="ExternalInput").ap(),
        "bq": nc.dram_tensor("bq", [EL], F32, kind="ExternalInput").ap(),
        "bk": nc.dram_tensor("bk", [EL], F32, kind="ExternalInput").ap(),
        "bv_bc": nc.dram_tensor("bv_bc", [128, EL], BF, kind="ExternalInput").ap(),
        "cos4": nc.dram_tensor("cos4", [128, S], BF, kind="ExternalInput").ap(),
        "sin4": nc.dram_tensor("sin4", [128, S], BF, kind="ExternalInput").ap(),
        "o": nc.dram_tensor("o", [E, S], BF, kind="ExternalOutput").ap(),
    }
    if variant == "causal":
        aps["utneg"] = nc.dram_tensor("utneg", [128, 128], BF,
                                      kind="ExternalInput").ap()
        aps["ident"] = nc.dram_tensor("ident", [128, 128], BF,
                                      kind="ExternalInput").ap()
    if variant == "masked":
        aps["mt"] = nc.dram_tensor("mt", [S, S], BF, kind="ExternalInput").ap()

    with tile.TileContext(nc) as tc, ExitStack() as ctx:
        if reps > 1:
            with tc.For_i(0, reps, 1, staggered_reset=True):
                _emit_body(nc, tc, ctx, aps, variant)
        else:
            _emit_body(nc, tc, ctx, aps, variant)
    nc.compile()
    _PROG_CACHE[key] = nc
    return nc


def _rope_tables():
    inv_freq = 1.0 / (10000.0 ** (np.arange(0, D, 2, dtype=np.float64) / D))
    pos = np.arange(S, dtype=np.float64)
    freqs = pos[:, None] * inv_freq[None, :]          # [S, 32]
    cos = np.cos(freqs).T.astype(np.float32)          # [32, S]
    sin = np.sin(freqs).T.astype(np.float32)
    cos4 = np.tile(cos, (4, 1)).astype(ml_dtypes.bfloat16)  # [128, S]
    sin4 = np.tile(sin, (4, 1)).astype(ml_dtypes.bfloat16)
    return cos4, sin4


def _qk_perm():
    # projection output column order: [r-rows heads 0-3 | i-rows heads 0-3 |
    #                                  r-rows heads 4-7 | i-rows heads 4-7]
    perm = []
    for g in range(2):
        for part in range(2):
            for h in range(4 * g, 4 * g + 4):
                for dd in range(32):
                    perm.append(h * D + part * 32 + dd)
    return np.array(perm)


def _prep_inputs(x, mask, Wq, bq, Wk, bk, Wv, bv, Wo, bo):
    x = np.asarray(x, dtype=np.float32)
    mask = np.asarray(mask).astype(bool)
    to_np = lambda a: np.asarray(a, dtype=np.float32)
    Wq, bq, Wk, bk = to_np(Wq), to_np(bq), to_np(Wk), to_np(bk)
    Wv, bv, Wo, bo = to_np(Wv), to_np(bv), to_np(Wo), to_np(bo)

    if mask.all():
        variant = "dense"
    elif np.array_equal(mask, np.tril(np.ones((S, S), dtype=bool))):
        variant = "causal"
    else:
        variant = "masked"

    cos4, sin4 = _rope_tables()
    perm = _qk_perm()
    bf = ml_dtypes.bfloat16

    in_maps = []
    common = {}
    if variant == "causal":
        jj = np.arange(128)
        common["utneg"] = np.where(jj[:, None] < jj[None, :], NEG,
                                   0.0).astype(bf)
        common["ident"] = np.eye(128).astype(bf)
    if variant == "masked":
        common["mt"] = mask.T.astype(bf)
    for c in range(NCORE):
        b, hh = c // 2, c % 2
        sl = slice(hh * EL, (hh + 1) * EL)
        m = {
            "xt": np.ascontiguousarray(x[b].T).astype(bf),
            "wq": Wq[:, sl][:, perm].astype(bf),
            "wk": Wk[:, sl][:, perm].astype(bf),
            "wv": Wv[:, sl].astype(bf),
            "wo": Wo[sl, :].astype(bf),
            "bq": np.ascontiguousarray(bq[sl][perm]),
            "bk": np.ascontiguousarray(bk[sl][perm]),
            "bv_bc": np.tile(bv[sl][None, :], (128, 1)).astype(bf),
            "cos4": cos4,
            "sin4": sin4,
        }
        m.update(common)
        in_maps.append(m)
    return variant, in_maps, bo


def kernel(x, mask, Wq, bq, Wk, bk, Wv, bv, Wo, bo):
    variant, in_maps, bo_np = _prep_inputs(x, mask, Wq, bq, Wk, bk, Wv, bv,
                                           Wo, bo)
    nc = _build_program(variant)
    res = None
    last_err = None
    for _attempt in range(3):
        try:
            res = run_bass_kernel_spmd(nc, in_maps, list(range(NCORE)))
            break
        except Exception as e:  # sporadic NRT device flakes: retry
            last_err = e
            import time as _time
            _time.sleep(3)
    if res is None:
        raise last_err
    out = np.empty((B, S, E), dtype=np.float32)
    for b in range(B):
        acc = (res.results[2 * b]["o"].astype(np.float32)
               + res.results[2 * b + 1]["o"].astype(np.float32))
        out[b] = acc.T + bo_np[None, :]
    return out
